# revision 54
# baseline (speedup 1.0000x reference)
"""MoE FFN (flylora + ERA) Trainium2 kernel.

Expert-parallel over 8 NeuronCores: core e holds expert e's weights and
processes the tokens routed to it (top-2 routing computed on host as part of
the sharding step). All heavy math — the three [*,1024]x[1024,2048]-class
matmuls per expert plus the LoRA chain and ERA activation — runs on device.

Device-side layout trick: everything is computed transposed ([feature, token]),
so every matmul's stationary operand comes straight from the natural weight
layout and no on-device transposes are needed:
    hT[I,C]  = basW[H,I] as lhsT tiles, xeT[H,C] as moving     (mm1, both branches)
    loT[R,C] = downW[H,R] as lhsT, xeT moving                   (lora down)
    hT      += SCALING*upW[R,I] as lhsT, loT moving             (lora up, same psum)
    yT[H,C]  = downProj[I,H] as lhsT, prodT[I,C] moving         (mm2)
Biases fold to one per-partition vector: cb = base_b + S*(up_b + down_b@up_w).
ERA(x) = gelu_tanh(x) + 0.1*softplus(x); softplus = Ln(1+Exp(x)) because the
deployed ACT tables have no Softplus entry. Activation instructions are
batched per table set (Exp / Ln / Gelu_apprx_tanh live in three different
sets) so only 3 ACT table loads happen. Up-branch and output evacuations run
on the Vector engine to keep ScalarE off the PSUM-recycle critical path, and
one shared 4-slot PSUM pool lets the PE run several tiles ahead of the
evacuations.
"""

import sys

sys.path.insert(0, "/opt/trn_rl_repo")

import numpy as np
import ml_dtypes

import concourse.bass as bass  # noqa: F401
import concourse.mybir as mybir
import concourse.tile as tile
from concourse.tile import add_dep_helper
from concourse import bacc
from concourse import bass_utils
from concourse.bass_utils import run_bass_kernel_spmd


def _ensure_trace_hooks():
    """bass_utils' trace path imports antenv.axon_hooks, which this image may
    lack; provide it (with the real NTFF hook when the axon .so supports it)
    so running under BASS_TRACE=1 degrades gracefully instead of crashing.
    Also keep profile artifacts local."""
    try:
        import antenv.axon_hooks  # noqa: F401
        return
    except ImportError:
        pass
    import types
    hook = None
    try:
        import contextlib
        import ctypes
        lib = ctypes.CDLL("/opt/axon/libaxon_pjrt.so")
        if hasattr(lib, "axon_start_nrt_profile"):
            lib.axon_start_nrt_profile.argtypes = [
                ctypes.POINTER(ctypes.c_int64), ctypes.c_size_t]
            lib.axon_start_nrt_profile.restype = ctypes.c_int64
            lib.axon_stop_nrt_profile.argtypes = [ctypes.c_char_p]
            lib.axon_stop_nrt_profile.restype = ctypes.c_int64

            @contextlib.contextmanager
            def hook(output_dir, device_ids):
                import jax
                jax.devices()
                if device_ids:
                    ids = (ctypes.c_int64 * len(device_ids))(*device_ids)
                    rc = lib.axon_start_nrt_profile(ids, len(device_ids))
                else:
                    rc = lib.axon_start_nrt_profile(None, 0)
                if rc != 0:
                    raise RuntimeError(f"axon_start_nrt_profile rc={rc}")
                try:
                    yield
                finally:
                    lib.axon_stop_nrt_profile(str(output_dir).encode())
    except Exception:
        hook = None
    mod = types.ModuleType("antenv.axon_hooks")
    mod.get_axon_ntff_profile_hook = lambda: hook
    mod.set_axon_ntff_profile_hook = lambda h: None
    try:
        import antenv
        antenv.axon_hooks = mod
    except ImportError:
        pass
    sys.modules["antenv.axon_hooks"] = mod
    bass_utils.upload_artifacts = lambda tmpdir: f"local:{tmpdir}"


_ensure_trace_hooks()

H = 1024
I = 2048
E = 8
TOP_K = 2
R = 16
SCALING = 32.0 / 16.0
GAMMA = 0.1
KH = H // 128   # 8  k-tiles for H contraction
MI = I // 128   # 16 m-tiles over I
MH = H // 128   # 8  m-tiles over H (mm2 output)
KI = I // 128   # 16 k-tiles for I contraction
GRP = 4         # m-tiles per mm1 weight DMA slab
DGRP = 2        # m-tiles per mm2 weight DMA slab
QMI = 4         # m-tiles per batched activation quarter

AF = mybir.ActivationFunctionType
DT = mybir.dt

_cache: dict = {}
_last_results = None  # BassKernelResults of the most recent run (for profiling)


def _build(C: int, cws: tuple):
    nch = len(cws)
    offs = [sum(cws[:j]) for j in range(nch)]
    nc = bacc.Bacc("TRN2", target_bir_lowering=False, debug=False, num_devices=8)

    d_xe = nc.dram_tensor("xe", [128, KH, C], DT.bfloat16, kind="ExternalInput").ap()
    d_gw = nc.dram_tensor("gw", [MI // GRP, 128, GRP * H], DT.bfloat16, kind="ExternalInput").ap()
    d_uw = nc.dram_tensor("uw", [MI // GRP, 128, GRP * H], DT.bfloat16, kind="ExternalInput").ap()
    d_dw = nc.dram_tensor("dw", [MH // DGRP, 128, DGRP * I], DT.bfloat16, kind="ExternalInput").ap()
    # constant blobs: lora-down weights (full 128p), lora-up weights (2R p),
    # and the f32 biases cbg | cbu | dbb
    d_c16a = nc.dram_tensor("c16a", [128, 2 * KH * R], DT.bfloat16, kind="ExternalInput").ap()
    d_c16b = nc.dram_tensor("c16b", [128, 2 * I], DT.bfloat16, kind="ExternalInput").ap()
    d_cf = nc.dram_tensor("cf", [128, 2 * MI + MH], DT.float32, kind="ExternalInput").ap()
    d_y = nc.dram_tensor("y", [MH, 128, C], DT.float32, kind="ExternalOutput").ap()

    from contextlib import ExitStack

    with tile.TileContext(nc) as tc, ExitStack() as ctx:
        consts = ctx.enter_context(tc.tile_pool(name="consts", bufs=1))
        big = ctx.enter_context(tc.tile_pool(name="big", bufs=1))
        wpool = ctx.enter_context(tc.tile_pool(name="wpool", bufs=3))
        dpool = ctx.enter_context(tc.tile_pool(name="dpool", bufs=2))
        ypool = ctx.enter_context(tc.tile_pool(name="ypool", bufs=3))
        psum_bufs = 4 if nch >= 2 else 8
        ps = ctx.enter_context(tc.tile_pool(name="ps", bufs=psum_bufs, space="PSUM"))

        # --- constant / input loads. xe lives as two independent tiles so
        # the lora-down k-loop starts while the second half is in flight ---
        KHa = KH // 2
        xe_a = consts.tile([128, KHa, C], DT.bfloat16)
        xe_b = consts.tile([128, KH - KHa, C], DT.bfloat16)
        c16a = consts.tile([128, 2 * KH * R], DT.bfloat16)
        c16b = consts.tile([128, 2 * I], DT.bfloat16)
        cf = consts.tile([128, 2 * MI + MH], DT.float32)
        nc.sync.dma_start(out=c16a, in_=d_c16a)
        nc.sync.dma_start(out=xe_a, in_=d_xe[:, :KHa, :])
        nc.sync.dma_start(out=xe_b, in_=d_xe[:, KHa:, :])
        # first gate weight slab ahead of the remaining constants: the base
        # matmuls are gated on it, while c16b/cf are only needed ~4us later
        wslab_g0 = wpool.tile([128, GRP * H], DT.bfloat16, tag="wslab")
        nc.sync.dma_start(out=wslab_g0, in_=d_gw[0])
        nc.sync.dma_start(out=c16b, in_=d_c16b)
        nc.sync.dma_start(out=cf, in_=d_cf)

        def xek(k):
            return xe_a[:, k, :] if k < KHa else xe_b[:, k - KHa, :]

        # lora-up lhsT blocks are full K=128 (rows beyond each branch's R
        # rows are zero, and lo rows 32..127 are zeroed once), so both
        # branches use the same moving operand `lo` and the lora-up matmuls
        # behave identically to base matmuls in the weight-load pipeline
        guw = c16b[:, 0:I]
        uuw = c16b[:, I:2 * I]
        lo = consts.tile([128, C], DT.bfloat16)
        nc.vector.memset(lo, 0.0)
        cbg = cf[:, 0:MI]
        cbu = cf[:, MI:2 * MI]
        dbb = cf[:, 2 * MI:2 * MI + MH]

        # --- lora down: both branches in one M=32 matmul chain:
        # loT[2R, C] rows 0..R-1 = gate, R..2R-1 = up (c16a packs gdw|udw
        # per k-tile) ---
        pl = ps.tile([2 * R, nch, 512], DT.float32, tag="acc")
        for k in range(KH):
            for j in range(nch):
                nc.tensor.matmul(
                    pl[:, j, :cws[j]],
                    c16a[:, k * 2 * R:(k + 1) * 2 * R],
                    xek(k)[:, offs[j]:offs[j] + cws[j]],
                    start=(k == 0),
                    stop=(k == KH - 1),
                    skip_group_check=True,
                )
        for j in range(nch):
            nc.vector.tensor_copy(lo[:2 * R, offs[j]:offs[j] + cws[j]],
                                  pl[:, j, :cws[j]])

        # --- big working buffers ---
        A = big.tile([128, MI, C], DT.float32)    # gate pre-act, then gelu(pre) in place
        Bb = big.tile([128, MI, C], DT.float32)   # exp(pre), then softplus in place
        U = big.tile([128, MI, C], DT.bfloat16)   # up branch (bias applied)
        GB = big.tile([128, MI, C], DT.bfloat16)  # ERA(gate)
        P = big.tile([128, MI, C], DT.bfloat16)   # gate*up, mm2 moving operand

        def mm1_pair(wslab, mi0, upw, m0):
            # two m-tiles' matmul streams interleaved: each LDWEIGHTS hides
            # under the ~2 matmuls of the other tile
            pg0 = ps.tile([128, nch, 512], DT.float32, tag="acc")
            pg1 = ps.tile([128, nch, 512], DT.float32, tag="acc")
            pgs = [pg0, pg1]
            for k in range(KH):
                for t in range(2):
                    mi = mi0 + t
                    for j in range(nch):
                        nc.tensor.matmul(
                            pgs[t][:, j, :cws[j]],
                            wslab[:, mi * H + k * 128: mi * H + (k + 1) * 128],
                            xek(k)[:, offs[j]:offs[j] + cws[j]],
                            start=(k == 0),
                            stop=False,
                            skip_group_check=True,
                        )
            # lora-up last; interleaved across the pair so each weight load
            # has another matmul to hide under
            for j in range(nch):
                for t in range(2):
                    m = m0 + t
                    nc.tensor.matmul(
                        pgs[t][:, j, :cws[j]],
                        upw[:, m * 128:(m + 1) * 128],
                        lo[:, offs[j]:offs[j] + cws[j]],
                        start=False,
                        stop=(j == nch - 1),
                        skip_group_check=True,
                    )
            return pgs

        def evac(engine_fn, pg, dst_row, bias_col):
            if len(set(cws)) == 1:
                engine_fn(dst_row.rearrange("p (j c) -> p j c", j=nch),
                          pg[:, :, :cws[0]], bias_col)
            else:
                for j in range(nch):
                    engine_fn(dst_row[:, offs[j]:offs[j] + cws[j]],
                              pg[:, j, :cws[j]], bias_col)

        # gate branch: ACT evac (Identity+bias) to A; Exp quarters batched in
        for g in range(MI // GRP):
            if g == 0:
                wslab = wslab_g0
            else:
                wslab = wpool.tile([128, GRP * H], DT.bfloat16, tag="wslab")
                nc.sync.dma_start(out=wslab, in_=d_gw[g])
            for mi in range(0, GRP, 2):
                m = g * GRP + mi
                pgs = mm1_pair(wslab, mi, guw, m)
                for t in range(2):
                    evac(lambda o, i, b: nc.scalar.activation(
                            o, i, AF.Identity, bias=b),
                         pgs[t], A[:, m + t, :], cbg[:, m + t:m + t + 1])
            if (g + 1) * GRP % QMI == 0:
                q0 = (g + 1) * GRP - QMI
                a_q = A[:, q0:q0 + QMI, :].rearrange("p m c -> p (m c)")
                b_q = Bb[:, q0:q0 + QMI, :].rearrange("p m c -> p (m c)")
                last_exp = nc.scalar.activation(b_q, a_q, AF.Exp)

        # softplus: all Ln ops contiguous (one table switch). Ordering deps
        # keep the scheduler from interleaving table sets (Tile isn't
        # table-set aware; each interleave would cost a ~1.3us table load).
        last_ln = None
        for q in range(MI // QMI):
            b_q = Bb[:, q * QMI:(q + 1) * QMI, :].rearrange("p m c -> p (m c)")
            last_ln = nc.scalar.activation(b_q, b_q, AF.Ln, bias=1.0)
            add_dep_helper(last_ln.ins, last_exp.ins, sync=False,
                           reason="ACT table-set batching: Ln after all Exp")

        # up branch: DVE evac (add bias, cast bf16) to U; per quarter finish
        # the gate path (Gelu on ACT, combine on DVE); the per-m product
        # directly follows each up evacuation so P trails by ~1 tile.
        for g in range(MI // GRP):
            wslab = wpool.tile([128, GRP * H], DT.bfloat16, tag="wslab")
            nc.sync.dma_start(out=wslab, in_=d_uw[g])
            if g * GRP % QMI == 0:
                q0 = g * GRP
                a_q = A[:, q0:q0 + QMI, :].rearrange("p m c -> p (m c)")
                gelu_i = nc.scalar.activation(a_q, a_q, AF.Gelu_apprx_tanh)
                add_dep_helper(gelu_i.ins, last_ln.ins, sync=False,
                               reason="ACT table-set batching: Gelu after all Ln")
            for mi in range(0, GRP, 2):
                m = g * GRP + mi
                pgs = mm1_pair(wslab, mi, uuw, m)
                for t in range(2):
                    # per-m combine, then a single fused op that evacuates the
                    # up-branch PSUM, adds its bias, and multiplies by the
                    # gate: P = (psum + cbu) * GB. Keeps every DVE op short
                    # and removes one full DVE pass per tile.
                    nc.vector.scalar_tensor_tensor(
                        GB[:, m + t, :], Bb[:, m + t, :], GAMMA, A[:, m + t, :],
                        mybir.AluOpType.mult, mybir.AluOpType.add,
                    )
                    if len(set(cws)) == 1:
                        nc.vector.scalar_tensor_tensor(
                            P[:, m + t, :].rearrange("p (j c) -> p j c", j=nch),
                            pgs[t][:, :, :cws[0]],
                            cbu[:, m + t:m + t + 1],
                            GB[:, m + t, :].rearrange("p (j c) -> p j c", j=nch),
                            mybir.AluOpType.add, mybir.AluOpType.mult,
                        )
                    else:
                        for j in range(nch):
                            sl = slice(offs[j], offs[j] + cws[j])
                            nc.vector.scalar_tensor_tensor(
                                P[:, m + t, sl], pgs[t][:, j, :cws[j]],
                                cbu[:, m + t:m + t + 1], GB[:, m + t, sl],
                                mybir.AluOpType.add, mybir.AluOpType.mult,
                            )

        # --- mm2: yT[H, C] = down_w.T @ prodT, accumulated over KI ---
        for g in range(MH // DGRP):
            dslab = dpool.tile([128, DGRP * I], DT.bfloat16)
            nc.sync.dma_start(out=dslab, in_=d_dw[g])
            py0 = ps.tile([128, nch, 512], DT.float32, tag="acc")
            py1 = ps.tile([128, nch, 512], DT.float32, tag="acc")
            pys = [py0, py1]
            for k in range(KI):
                for mi in range(DGRP):
                    for j in range(nch):
                        nc.tensor.matmul(
                            pys[mi][:, j, :cws[j]],
                            dslab[:, mi * I + k * 128: mi * I + (k + 1) * 128],
                            P[:, k, offs[j]:offs[j] + cws[j]],
                            start=(k == 0),
                            stop=(k == KI - 1),
                            skip_group_check=True,
                        )
            for mi in range(DGRP):
                m = g * DGRP + mi
                yt = ypool.tile([128, C], DT.float32)
                evac(nc.vector.tensor_scalar_add, pys[mi], yt, dbb[:, m:m + 1])
                nc.sync.dma_start(out=d_y[m], in_=yt)

    nc.compile()
    return nc


def _pack_inputs(e, xf_b16, toks, C, w):
    """Per-core input map for expert e; token block already chosen."""
    n = len(toks)
    xe = np.zeros((H, C), dtype=ml_dtypes.bfloat16)
    if n:
        xe[:, :n] = xf_b16[toks].T
    xe = np.ascontiguousarray(
        xe.reshape(KH, 128, C).transpose(1, 0, 2))  # [128, KH, C]

    def pack_mm1(wt):  # [H, I] -> [MI//GRP, 128, GRP*H]
        t = wt.reshape(KH, 128, MI, 128).transpose(2, 1, 0, 3).reshape(MI, 128, H)
        return np.ascontiguousarray(
            t.reshape(MI // GRP, GRP, 128, H).transpose(0, 2, 1, 3)
            .reshape(MI // GRP, 128, GRP * H))

    def pack_mm2(wt):  # [I, H] -> [MH//DGRP, 128, DGRP*I]
        t = wt.reshape(KI, 128, MH, 128).transpose(2, 1, 0, 3).reshape(MH, 128, I)
        return np.ascontiguousarray(
            t.reshape(MH // DGRP, DGRP, 128, I).transpose(0, 2, 1, 3)
            .reshape(MH // DGRP, 128, DGRP * I))

    b16 = ml_dtypes.bfloat16
    gw = pack_mm1(w["gate_base_w"][e].astype(b16))
    uw = pack_mm1(w["up_base_w"][e].astype(b16))
    dw = pack_mm2(w["down_w"][e].astype(b16))

    # per k-tile: [gdw_k | udw_k] so one M=32 matmul computes both branches
    c16a = np.concatenate([
        w["gate_down_w"][e].astype(b16).reshape(KH, 128, R),
        w["up_down_w"][e].astype(b16).reshape(KH, 128, R),
    ], axis=2).transpose(1, 0, 2).reshape(128, 2 * KH * R)
    c16b = np.zeros((128, 2 * I), dtype=b16)
    c16b[0:R, 0:I] = (SCALING * w["gate_up_w"][e]).astype(b16)
    c16b[R:2 * R, I:2 * I] = (SCALING * w["up_up_w"][e]).astype(b16)

    cbg = (w["gate_base_b"][e].astype(np.float64)
           + SCALING * (w["gate_up_b"][e].astype(np.float64)
                        + w["gate_down_b"][e].astype(np.float64)
                        @ w["gate_up_w"][e].astype(np.float64))).astype(np.float32)
    cbu = (w["up_base_b"][e].astype(np.float64)
           + SCALING * (w["up_up_b"][e].astype(np.float64)
                        + w["up_down_b"][e].astype(np.float64)
                        @ w["up_up_w"][e].astype(np.float64))).astype(np.float32)
    cf = np.concatenate([
        cbg.reshape(MI, 128).T,
        cbu.reshape(MI, 128).T,
        w["down_b"][e].astype(np.float32).reshape(MH, 128).T,
    ], axis=1)
    return {
        "xe": xe, "gw": gw, "uw": uw, "dw": dw,
        "c16a": c16a, "c16b": np.ascontiguousarray(c16b),
        "cf": np.ascontiguousarray(cf),
    }


def kernel(**inputs):
    global _last_results
    w = {k: np.asarray(v) for k, v in inputs.items()}
    x = w["x"]
    b, s, _ = x.shape
    T = b * s
    xf = x.reshape(T, H).astype(np.float32)

    # --- router (host; this determines the sharding) ---
    logits = xf @ w["router_w"].astype(np.float32) + w["router_b"].astype(np.float32)
    mx = logits.max(-1, keepdims=True)
    ex = np.exp(logits - mx)
    probs = ex / ex.sum(-1, keepdims=True)
    ti = np.argsort(-probs, axis=-1, kind="stable")[:, :TOP_K]
    tp = np.take_along_axis(probs, ti, axis=-1)
    tw = tp / tp.sum(-1, keepdims=True)

    p_mean = probs.mean(axis=0)
    f = np.bincount(ti.ravel(), minlength=E).astype(np.float32) / (T * TOP_K)
    aux_loss = np.float32(E * np.sum(f * p_mean))

    toks_all, wts_all = [], []
    for e in range(E):
        t_idx, slot = np.nonzero(ti == e)
        toks_all.append(t_idx)
        wts_all.append(tw[t_idx, slot].astype(np.float32))
    counts = np.array([len(t) for t in toks_all])

    xf_b16 = xf.astype(ml_dtypes.bfloat16)
    out_f = np.zeros((T, H), dtype=np.float32)

    # token blocks of at most 1024 per expert per kernel launch
    CB = 1024
    n_blocks = max(1, int(-(-counts.max() // CB)))
    for blk in range(n_blocks):
        blk_toks = [t[blk * CB:(blk + 1) * CB] for t in toks_all]
        blk_max = max(len(t) for t in blk_toks)
        if blk_max == 0:
            continue
        C = max(128, -(-blk_max // 8) * 8)
        # chunk widths (each <= 512 = one fp32 PSUM bank per matmul).
        # The last chunk is kept >= ~330 columns so its matmul fully hides
        # the next LDWEIGHTS (~137ns) behind its streaming time.
        if C <= 512:
            cws = (C,)
        else:
            cws = (C // 2, C - C // 2)

        key = (C, cws)
        if key not in _cache:
            _cache[key] = _build(C, cws)
        nc = _cache[key]

        in_maps = [
            _pack_inputs(e, xf_b16, blk_toks[e], C, w) for e in range(E)
        ]
        res = run_bass_kernel_spmd(nc, in_maps, core_ids=list(range(8)))
        _last_results = res

        for e in range(E):
            n = len(blk_toks[e])
            if n == 0:
                continue
            y = res.results[e]["y"].reshape(H, C)  # [H, C]; row h = k*128+p
            wgt = wts_all[e][blk * CB: blk * CB + n]
            out_f[blk_toks[e]] += wgt[:, None] * y[:, :n].T

    return out_f.reshape(b, s, H), aux_loss


# revision 55
# speedup vs baseline: 1.0907x; 1.0907x over previous
"""MoE FFN (flylora + ERA) Trainium2 kernel.

Expert-parallel over 8 NeuronCores: core e holds expert e's weights and
processes the tokens routed to it (top-2 routing computed on host as part of
the sharding step). All heavy math — the three [*,1024]x[1024,2048]-class
matmuls per expert plus the LoRA chain and ERA activation — runs on device.

Device-side layout trick: everything is computed transposed ([feature, token]),
so every matmul's stationary operand comes straight from the natural weight
layout and no on-device transposes are needed:
    hT[I,C]  = basW[H,I] as lhsT tiles, xeT[H,C] as moving     (mm1, both branches)
    loT[R,C] = downW[H,R] as lhsT, xeT moving                   (lora down)
    hT      += SCALING*upW[R,I] as lhsT, loT moving             (lora up, same psum)
    yT[H,C]  = downProj[I,H] as lhsT, prodT[I,C] moving         (mm2)
Biases fold to one per-partition vector: cb = base_b + S*(up_b + down_b@up_w).
ERA(x) = gelu_tanh(x) + 0.1*softplus(x); softplus = Ln(1+Exp(x)) because the
deployed ACT tables have no Softplus entry. Activation instructions are
batched per table set (Exp / Ln / Gelu_apprx_tanh live in three different
sets) so only 3 ACT table loads happen. Up-branch and output evacuations run
on the Vector engine to keep ScalarE off the PSUM-recycle critical path, and
one shared 4-slot PSUM pool lets the PE run several tiles ahead of the
evacuations.
"""

import sys

sys.path.insert(0, "/opt/trn_rl_repo")

import numpy as np
import ml_dtypes

import concourse.bass as bass  # noqa: F401
import concourse.mybir as mybir
import concourse.tile as tile
from concourse.tile import add_dep_helper
from concourse import bacc
from concourse import bass_utils
from concourse.bass_utils import run_bass_kernel_spmd


def _ensure_trace_hooks():
    """bass_utils' trace path imports antenv.axon_hooks, which this image may
    lack; provide it (with the real NTFF hook when the axon .so supports it)
    so running under BASS_TRACE=1 degrades gracefully instead of crashing.
    Also keep profile artifacts local."""
    try:
        import antenv.axon_hooks  # noqa: F401
        return
    except ImportError:
        pass
    import types
    hook = None
    try:
        import contextlib
        import ctypes
        lib = ctypes.CDLL("/opt/axon/libaxon_pjrt.so")
        if hasattr(lib, "axon_start_nrt_profile"):
            lib.axon_start_nrt_profile.argtypes = [
                ctypes.POINTER(ctypes.c_int64), ctypes.c_size_t]
            lib.axon_start_nrt_profile.restype = ctypes.c_int64
            lib.axon_stop_nrt_profile.argtypes = [ctypes.c_char_p]
            lib.axon_stop_nrt_profile.restype = ctypes.c_int64

            @contextlib.contextmanager
            def hook(output_dir, device_ids):
                import jax
                jax.devices()
                if device_ids:
                    ids = (ctypes.c_int64 * len(device_ids))(*device_ids)
                    rc = lib.axon_start_nrt_profile(ids, len(device_ids))
                else:
                    rc = lib.axon_start_nrt_profile(None, 0)
                if rc != 0:
                    raise RuntimeError(f"axon_start_nrt_profile rc={rc}")
                try:
                    yield
                finally:
                    lib.axon_stop_nrt_profile(str(output_dir).encode())
    except Exception:
        hook = None
    mod = types.ModuleType("antenv.axon_hooks")
    mod.get_axon_ntff_profile_hook = lambda: hook
    mod.set_axon_ntff_profile_hook = lambda h: None
    try:
        import antenv
        antenv.axon_hooks = mod
    except ImportError:
        pass
    sys.modules["antenv.axon_hooks"] = mod
    bass_utils.upload_artifacts = lambda tmpdir: f"local:{tmpdir}"


_ensure_trace_hooks()

H = 1024
I = 2048
E = 8
TOP_K = 2
R = 16
SCALING = 32.0 / 16.0
GAMMA = 0.1
KH = H // 128   # 8  k-tiles for H contraction
MI = I // 128   # 16 m-tiles over I
MH = H // 128   # 8  m-tiles over H (mm2 output)
KI = I // 128   # 16 k-tiles for I contraction
GRP = 4         # m-tiles per mm1 weight DMA slab
DGRP = 2        # m-tiles per mm2 weight DMA slab
QMI = 4         # m-tiles per batched activation quarter

AF = mybir.ActivationFunctionType
DT = mybir.dt

_cache: dict = {}
_last_results = None  # BassKernelResults of the most recent run (for profiling)


def _build(C: int, cws: tuple):
    nch = len(cws)
    offs = [sum(cws[:j]) for j in range(nch)]
    nc = bacc.Bacc("TRN2", target_bir_lowering=False, debug=False, num_devices=8)

    d_xe = nc.dram_tensor("xe", [128, KH, C], DT.bfloat16, kind="ExternalInput").ap()
    d_gw = nc.dram_tensor("gw", [MI // GRP, 128, GRP * H], DT.bfloat16, kind="ExternalInput").ap()
    d_uw = nc.dram_tensor("uw", [MI // GRP, 128, GRP * H], DT.bfloat16, kind="ExternalInput").ap()
    d_dw = nc.dram_tensor("dw", [MH // DGRP, 128, DGRP * I], DT.bfloat16, kind="ExternalInput").ap()
    # constant blobs: lora-down weights (full 128p), lora-up weights (2R p),
    # and the f32 biases cbg | cbu | dbb
    d_c16a = nc.dram_tensor("c16a", [128, 2 * KH * R], DT.bfloat16, kind="ExternalInput").ap()
    d_c16b = nc.dram_tensor("c16b", [128, 2 * I], DT.bfloat16, kind="ExternalInput").ap()
    d_cf = nc.dram_tensor("cf", [128, 2 * MI + MH], DT.float32, kind="ExternalInput").ap()
    d_y = nc.dram_tensor("y", [MH, 128, C], DT.float32, kind="ExternalOutput").ap()

    from contextlib import ExitStack

    with tile.TileContext(nc) as tc, ExitStack() as ctx:
        consts = ctx.enter_context(tc.tile_pool(name="consts", bufs=1))
        big = ctx.enter_context(tc.tile_pool(name="big", bufs=1))
        wpool = ctx.enter_context(tc.tile_pool(name="wpool", bufs=3))
        dpool = ctx.enter_context(tc.tile_pool(name="dpool", bufs=2))
        ypool = ctx.enter_context(tc.tile_pool(name="ypool", bufs=3))
        psum_bufs = 4 if nch >= 2 else 8
        ps = ctx.enter_context(tc.tile_pool(name="ps", bufs=psum_bufs, space="PSUM"))

        # --- constant / input loads. xe lives as two independent tiles so
        # the lora-down k-loop starts while the second half is in flight ---
        KHa = KH // 2
        xe_a = consts.tile([128, KHa, C], DT.bfloat16)
        xe_b = consts.tile([128, KH - KHa, C], DT.bfloat16)
        c16a = consts.tile([128, 2 * KH * R], DT.bfloat16)
        c16b = consts.tile([128, 2 * I], DT.bfloat16)
        cf = consts.tile([128, 2 * MI + MH], DT.float32)
        nc.sync.dma_start(out=c16a, in_=d_c16a)
        nc.sync.dma_start(out=xe_a, in_=d_xe[:, :KHa, :])
        nc.sync.dma_start(out=xe_b, in_=d_xe[:, KHa:, :])
        # first gate weight slab ahead of the remaining constants: the base
        # matmuls are gated on it, while c16b/cf are only needed ~4us later
        wslab_g0 = wpool.tile([128, GRP * H], DT.bfloat16, tag="wslab")
        nc.sync.dma_start(out=wslab_g0, in_=d_gw[0])
        nc.sync.dma_start(out=c16b, in_=d_c16b)
        nc.sync.dma_start(out=cf, in_=d_cf)

        def xek(k):
            return xe_a[:, k, :] if k < KHa else xe_b[:, k - KHa, :]

        # lora-up lhsT blocks are full K=128 (rows beyond each branch's R
        # rows are zero, and lo rows 32..127 are zeroed once), so both
        # branches use the same moving operand `lo` and the lora-up matmuls
        # behave identically to base matmuls in the weight-load pipeline
        guw = c16b[:, 0:I]
        uuw = c16b[:, I:2 * I]
        lo = consts.tile([128, C], DT.bfloat16)
        nc.vector.memset(lo, 0.0)
        cbg = cf[:, 0:MI]
        cbu = cf[:, MI:2 * MI]
        dbb = cf[:, 2 * MI:2 * MI + MH]

        # --- lora down: both branches in one M=32 matmul chain:
        # loT[2R, C] rows 0..R-1 = gate, R..2R-1 = up (c16a packs gdw|udw
        # per k-tile) ---
        pl = ps.tile([2 * R, nch, 512], DT.float32, tag="acc")
        for k in range(KH):
            for j in range(nch):
                nc.tensor.matmul(
                    pl[:, j, :cws[j]],
                    c16a[:, k * 2 * R:(k + 1) * 2 * R],
                    xek(k)[:, offs[j]:offs[j] + cws[j]],
                    start=(k == 0),
                    stop=(k == KH - 1),
                    skip_group_check=True,
                )
        for j in range(nch):
            nc.vector.tensor_copy(lo[:2 * R, offs[j]:offs[j] + cws[j]],
                                  pl[:, j, :cws[j]])

        # --- big working buffers ---
        A = big.tile([128, MI, C], DT.float32)    # gate pre-act, then gelu(pre) in place
        Bb = big.tile([128, MI, C], DT.float32)   # exp(pre), then softplus in place
        U = big.tile([128, MI, C], DT.bfloat16)   # up branch (bias applied)
        GB = big.tile([128, MI, C], DT.bfloat16)  # ERA(gate)
        P = big.tile([128, MI, C], DT.bfloat16)   # gate*up, mm2 moving operand

        def mm1_pair(wslab, mi0, upw, m0):
            # two m-tiles' matmul streams interleaved: each LDWEIGHTS hides
            # under the ~2 matmuls of the other tile
            pg0 = ps.tile([128, nch, 512], DT.float32, tag="acc")
            pg1 = ps.tile([128, nch, 512], DT.float32, tag="acc")
            pgs = [pg0, pg1]
            for k in range(KH):
                for t in range(2):
                    mi = mi0 + t
                    for j in range(nch):
                        nc.tensor.matmul(
                            pgs[t][:, j, :cws[j]],
                            wslab[:, mi * H + k * 128: mi * H + (k + 1) * 128],
                            xek(k)[:, offs[j]:offs[j] + cws[j]],
                            start=(k == 0),
                            stop=False,
                            skip_group_check=True,
                        )
            # lora-up last; interleaved across the pair so each weight load
            # has another matmul to hide under
            for j in range(nch):
                for t in range(2):
                    m = m0 + t
                    nc.tensor.matmul(
                        pgs[t][:, j, :cws[j]],
                        upw[:, m * 128:(m + 1) * 128],
                        lo[:, offs[j]:offs[j] + cws[j]],
                        start=False,
                        stop=(j == nch - 1),
                        skip_group_check=True,
                    )
            return pgs

        def evac(engine_fn, pg, dst_row, bias_col):
            if len(set(cws)) == 1:
                engine_fn(dst_row.rearrange("p (j c) -> p j c", j=nch),
                          pg[:, :, :cws[0]], bias_col)
            else:
                for j in range(nch):
                    engine_fn(dst_row[:, offs[j]:offs[j] + cws[j]],
                              pg[:, j, :cws[j]], bias_col)

        # gate branch: ACT evac (Identity+bias) to A; Exp quarters batched in
        for g in range(MI // GRP):
            if g == 0:
                wslab = wslab_g0
            else:
                wslab = wpool.tile([128, GRP * H], DT.bfloat16, tag="wslab")
                nc.sync.dma_start(out=wslab, in_=d_gw[g])
            for mi in range(0, GRP, 2):
                m = g * GRP + mi
                pgs = mm1_pair(wslab, mi, guw, m)
                for t in range(2):
                    evac(lambda o, i, b: nc.scalar.activation(
                            o, i, AF.Identity, bias=b),
                         pgs[t], A[:, m + t, :], cbg[:, m + t:m + t + 1])
            if (g + 1) * GRP % QMI == 0:
                q0 = (g + 1) * GRP - QMI
                a_q = A[:, q0:q0 + QMI, :].rearrange("p m c -> p (m c)")
                b_q = Bb[:, q0:q0 + QMI, :].rearrange("p m c -> p (m c)")
                last_exp = nc.scalar.activation(b_q, a_q, AF.Exp)

        # softplus: all Ln ops contiguous (one table switch). Ordering deps
        # keep the scheduler from interleaving table sets (Tile isn't
        # table-set aware; each interleave would cost a ~1.3us table load).
        last_ln = None
        for q in range(MI // QMI):
            b_q = Bb[:, q * QMI:(q + 1) * QMI, :].rearrange("p m c -> p (m c)")
            last_ln = nc.scalar.activation(b_q, b_q, AF.Ln, bias=1.0)
            add_dep_helper(last_ln.ins, last_exp.ins, sync=False,
                           reason="ACT table-set batching: Ln after all Exp")

        # up branch: DVE evac (add bias, cast bf16) to U; per quarter finish
        # the gate path (Gelu on ACT, combine on DVE); the per-m product
        # directly follows each up evacuation so P trails by ~1 tile.
        for g in range(MI // GRP):
            wslab = wpool.tile([128, GRP * H], DT.bfloat16, tag="wslab")
            nc.sync.dma_start(out=wslab, in_=d_uw[g])
            if g * GRP % QMI == 0:
                q0 = g * GRP
                a_q = A[:, q0:q0 + QMI, :].rearrange("p m c -> p (m c)")
                gelu_i = nc.scalar.activation(a_q, a_q, AF.Gelu_apprx_tanh)
                add_dep_helper(gelu_i.ins, last_ln.ins, sync=False,
                               reason="ACT table-set batching: Gelu after all Ln")
            for mi in range(0, GRP, 2):
                m = g * GRP + mi
                pgs = mm1_pair(wslab, mi, uuw, m)
                for t in range(2):
                    # per-m combine keeps every DVE op short (~0.7us) so the
                    # PSUM-freeing evacuations are never queued behind a
                    # multi-us burst
                    evac(nc.vector.tensor_scalar_add, pgs[t],
                         U[:, m + t, :], cbu[:, m + t:m + t + 1])
                    nc.vector.scalar_tensor_tensor(
                        GB[:, m + t, :], Bb[:, m + t, :], GAMMA, A[:, m + t, :],
                        mybir.AluOpType.mult, mybir.AluOpType.add,
                    )
                    nc.vector.tensor_mul(P[:, m + t, :], GB[:, m + t, :],
                                         U[:, m + t, :])

        # --- mm2: yT[H, C] = down_w.T @ prodT, accumulated over KI ---
        for g in range(MH // DGRP):
            dslab = dpool.tile([128, DGRP * I], DT.bfloat16)
            nc.sync.dma_start(out=dslab, in_=d_dw[g])
            py0 = ps.tile([128, nch, 512], DT.float32, tag="acc")
            py1 = ps.tile([128, nch, 512], DT.float32, tag="acc")
            pys = [py0, py1]
            for k in range(KI):
                for mi in range(DGRP):
                    for j in range(nch):
                        nc.tensor.matmul(
                            pys[mi][:, j, :cws[j]],
                            dslab[:, mi * I + k * 128: mi * I + (k + 1) * 128],
                            P[:, k, offs[j]:offs[j] + cws[j]],
                            start=(k == 0),
                            stop=(k == KI - 1),
                            skip_group_check=True,
                        )
            for mi in range(DGRP):
                m = g * DGRP + mi
                yt = ypool.tile([128, C], DT.float32)
                evac(nc.vector.tensor_scalar_add, pys[mi], yt, dbb[:, m:m + 1])
                nc.sync.dma_start(out=d_y[m], in_=yt)

    nc.compile()
    return nc


def _pack_inputs(e, xf_b16, toks, C, w):
    """Per-core input map for expert e; token block already chosen."""
    n = len(toks)
    xe = np.zeros((H, C), dtype=ml_dtypes.bfloat16)
    if n:
        xe[:, :n] = xf_b16[toks].T
    xe = np.ascontiguousarray(
        xe.reshape(KH, 128, C).transpose(1, 0, 2))  # [128, KH, C]

    def pack_mm1(wt):  # [H, I] -> [MI//GRP, 128, GRP*H]
        t = wt.reshape(KH, 128, MI, 128).transpose(2, 1, 0, 3).reshape(MI, 128, H)
        return np.ascontiguousarray(
            t.reshape(MI // GRP, GRP, 128, H).transpose(0, 2, 1, 3)
            .reshape(MI // GRP, 128, GRP * H))

    def pack_mm2(wt):  # [I, H] -> [MH//DGRP, 128, DGRP*I]
        t = wt.reshape(KI, 128, MH, 128).transpose(2, 1, 0, 3).reshape(MH, 128, I)
        return np.ascontiguousarray(
            t.reshape(MH // DGRP, DGRP, 128, I).transpose(0, 2, 1, 3)
            .reshape(MH // DGRP, 128, DGRP * I))

    b16 = ml_dtypes.bfloat16
    gw = pack_mm1(w["gate_base_w"][e].astype(b16))
    uw = pack_mm1(w["up_base_w"][e].astype(b16))
    dw = pack_mm2(w["down_w"][e].astype(b16))

    # per k-tile: [gdw_k | udw_k] so one M=32 matmul computes both branches
    c16a = np.concatenate([
        w["gate_down_w"][e].astype(b16).reshape(KH, 128, R),
        w["up_down_w"][e].astype(b16).reshape(KH, 128, R),
    ], axis=2).transpose(1, 0, 2).reshape(128, 2 * KH * R)
    c16b = np.zeros((128, 2 * I), dtype=b16)
    c16b[0:R, 0:I] = (SCALING * w["gate_up_w"][e]).astype(b16)
    c16b[R:2 * R, I:2 * I] = (SCALING * w["up_up_w"][e]).astype(b16)

    cbg = (w["gate_base_b"][e].astype(np.float64)
           + SCALING * (w["gate_up_b"][e].astype(np.float64)
                        + w["gate_down_b"][e].astype(np.float64)
                        @ w["gate_up_w"][e].astype(np.float64))).astype(np.float32)
    cbu = (w["up_base_b"][e].astype(np.float64)
           + SCALING * (w["up_up_b"][e].astype(np.float64)
                        + w["up_down_b"][e].astype(np.float64)
                        @ w["up_up_w"][e].astype(np.float64))).astype(np.float32)
    cf = np.concatenate([
        cbg.reshape(MI, 128).T,
        cbu.reshape(MI, 128).T,
        w["down_b"][e].astype(np.float32).reshape(MH, 128).T,
    ], axis=1)
    return {
        "xe": xe, "gw": gw, "uw": uw, "dw": dw,
        "c16a": c16a, "c16b": np.ascontiguousarray(c16b),
        "cf": np.ascontiguousarray(cf),
    }


def kernel(**inputs):
    global _last_results
    w = {k: np.asarray(v) for k, v in inputs.items()}
    x = w["x"]
    b, s, _ = x.shape
    T = b * s
    xf = x.reshape(T, H).astype(np.float32)

    # --- router (host; this determines the sharding) ---
    logits = xf @ w["router_w"].astype(np.float32) + w["router_b"].astype(np.float32)
    mx = logits.max(-1, keepdims=True)
    ex = np.exp(logits - mx)
    probs = ex / ex.sum(-1, keepdims=True)
    ti = np.argsort(-probs, axis=-1, kind="stable")[:, :TOP_K]
    tp = np.take_along_axis(probs, ti, axis=-1)
    tw = tp / tp.sum(-1, keepdims=True)

    p_mean = probs.mean(axis=0)
    f = np.bincount(ti.ravel(), minlength=E).astype(np.float32) / (T * TOP_K)
    aux_loss = np.float32(E * np.sum(f * p_mean))

    toks_all, wts_all = [], []
    for e in range(E):
        t_idx, slot = np.nonzero(ti == e)
        toks_all.append(t_idx)
        wts_all.append(tw[t_idx, slot].astype(np.float32))
    counts = np.array([len(t) for t in toks_all])

    xf_b16 = xf.astype(ml_dtypes.bfloat16)
    out_f = np.zeros((T, H), dtype=np.float32)

    # token blocks of at most 1024 per expert per kernel launch
    CB = 1024
    n_blocks = max(1, int(-(-counts.max() // CB)))
    for blk in range(n_blocks):
        blk_toks = [t[blk * CB:(blk + 1) * CB] for t in toks_all]
        blk_max = max(len(t) for t in blk_toks)
        if blk_max == 0:
            continue
        C = max(128, -(-blk_max // 8) * 8)
        # chunk widths (each <= 512 = one fp32 PSUM bank per matmul).
        # The last chunk is kept >= ~330 columns so its matmul fully hides
        # the next LDWEIGHTS (~137ns) behind its streaming time.
        if C <= 512:
            cws = (C,)
        else:
            cws = (C // 2, C - C // 2)

        key = (C, cws)
        if key not in _cache:
            _cache[key] = _build(C, cws)
        nc = _cache[key]

        in_maps = [
            _pack_inputs(e, xf_b16, blk_toks[e], C, w) for e in range(E)
        ]
        res = run_bass_kernel_spmd(nc, in_maps, core_ids=list(range(8)))
        _last_results = res

        for e in range(E):
            n = len(blk_toks[e])
            if n == 0:
                continue
            y = res.results[e]["y"].reshape(H, C)  # [H, C]; row h = k*128+p
            wgt = wts_all[e][blk * CB: blk * CB + n]
            out_f[blk_toks[e]] += wgt[:, None] * y[:, :n].T

    return out_f.reshape(b, s, H), aux_loss
